# revision 27
# baseline (speedup 1.0000x reference)
"""Trainium2 Bass kernel for nn_Attention_86672440033867 (relative-position attention).

Sharding: head-parallel over 8 NeuronCores (1 head per core, all 16 batches).

v2 design (PE-array tiling + batch-level software pipeline):
  - QKV pass: stationary [wq|wk|wq|wk] -> psum [q,k,q,k] rows -> QK4 sbuf.
    4 cross-partition DMAs/batch build K2 (k on strips 0,2) and Q2 (q on
    strips 1,3) so every 32-row strip holds a (K-stationary, Q-moving) pair.
  - V natural: stationary x^T chunks, moving w_v -> V in [token, d] layout
    (no PE transposes of v).
  - QK^T: 4x row-tiled (tile_position auto from base partitions): 4 key
    chunks computed concurrently, contraction=32 each.  ~3x faster.
  - exp on ACT from psum [128, 4x256] (bias applied multiplicatively:
    P = exp(s*S) * exp(B), exp(B) host-precomputed, SBUF-resident).
  - multiply es*expb on DVE (2x bf16), 1 in 4 tiles on GPSIMD.
  - AV: 2x col-tiled (64-col groups): queries split in two 128-chunks, V
    carries a ones column -> softmax denominators land at psum rows 32/96.
  - denominators -> natural layout via 2 tiny PE transposes/unit -> DVE
    reciprocal (straight from psum).
  - out-proj: 2x row-tiled (strips 0 and 2 of the normalized-later OT).
  - normalize fused into the psum evacuation (tensor_scalar_mul by recip),
    split DVE/ACT; host sums the 8 partial projections and adds b_out.
"""
import numpy as np
import ml_dtypes
from collections import deque
from contextlib import ExitStack, nullcontext

import concourse.bass as bass
import concourse.mybir as mybir
import concourse.tile as tile
from concourse import bacc
from concourse.bass_utils import run_bass_kernel_spmd

BF16 = mybir.dt.bfloat16
F32 = mybir.dt.float32

HEADS = 8
D = 32          # head dim
INP = 384
OUP = 384
SCALE = D ** -0.5
AF = mybir.ActivationFunctionType


def build_kernel(NB=16, N=1024, num_devices=8, loop_k=0, row4=True, col2=True,
                 nstrips=None, ablate=(), vmode="mm", avw=33, lag=3, qkvpd=0):
    """Build the per-core Bass module. NB = total batches, N = tokens/batch."""
    assert N % 256 == 0
    NJC = N // 128          # key chunks (128) per batch
    IH = 256                # queries per attention unit
    NIH = N // IH           # units per batch
    TOK = NB * N

    nc = bacc.Bacc("TRN2", target_bir_lowering=False, num_devices=num_devices)

    xt_d = nc.dram_tensor("xt", [INP, TOK], BF16, kind="ExternalInput")
    wqkv4_d = nc.dram_tensor("wqkv4", [3, 128, 128], BF16, kind="ExternalInput")
    wv3_d = nc.dram_tensor("wv3", [3, 128, 32], BF16, kind="ExternalInput")
    wout4_d = nc.dram_tensor("wout4", [128, OUP], BF16, kind="ExternalInput")
    expb_d = nc.dram_tensor("expb", [128, NJC, N], BF16, kind="ExternalInput")
    ident_d = nc.dram_tensor("ident", [128, 32], BF16, kind="ExternalInput")
    outp_d = nc.dram_tensor("outp", [TOK, OUP], BF16, kind="ExternalOutput")

    with tile.TileContext(nc) as tc, ExitStack() as ctx:
        const = ctx.enter_context(tc.tile_pool(name="const", bufs=1))
        big = ctx.enter_context(tc.tile_pool(name="big", bufs=1))

        wqkv_sb = const.tile([128, 3, 128], BF16)
        wv_sb = const.tile([128, 3, 32], BF16)
        wout_sb = const.tile([128, OUP], BF16)
        ident_sb = const.tile([128, 32], BF16)
        expb_sb = const.tile([128, NJC, N], BF16)
        for kc in range(3):
            nc.sync.dma_start(wqkv_sb[:, kc, :], wqkv4_d.ap()[kc])
            nc.sync.dma_start(wv_sb[:, kc, :], wv3_d.ap()[kc])
        nc.sync.dma_start(wout_sb[:], wout4_d.ap())
        nc.sync.dma_start(ident_sb[:], ident_d.ap())
        nc.sync.dma_start(expb_sb[:], expb_d.ap())

        # Resident activation layouts
        QK4 = big.tile([128, TOK], BF16)     # strips 0,2: qT ; strips 1,3: kT
        K2 = big.tile([128, TOK], BF16)      # kT on strips 0,2 (DMA-replicated)
        Q2 = big.tile([128, TOK], BF16)      # qT on strips 1,3 (DMA-replicated)
        V_sb = big.tile([128, NB * NJC * 64], BF16)  # v natural + ones cols
        recip_nat = big.tile([128, NB * NIH * 2], F32)

        nc.gpsimd.memset(V_sb[:], 1.0)  # ones column pre-fill

        xt_pool = ctx.enter_context(tc.tile_pool(name="xt", bufs=6))
        es_pool = ctx.enter_context(tc.tile_pool(name="es", bufs=4))
        pt_pool = ctx.enter_context(tc.tile_pool(name="pt", bufs=6))
        ot_pool = ctx.enter_context(tc.tile_pool(name="ot", bufs=4))
        ob_pool = ctx.enter_context(tc.tile_pool(name="ob", bufs=3))

        vv = V_sb[:].rearrange("p (b j e) -> p b j e", j=NJC, e=64)

        loopB = tc.For_i(0, loop_k, 1) if loop_k else nullcontext()
        with tc.tile_pool(name="pd", bufs=2, space="PSUM") as pd_pool, \
             tc.tile_pool(name="pa", bufs=2, space="PSUM") as pa_pool, \
             tc.tile_pool(name="po", bufs=2, space="PSUM") as po_pool, loopB:

            def stage_a(b):
                """QKV projections, V-natural, replication DMAs for batch b."""
                for t in range(N // 512):
                    xt_t = xt_pool.tile([128, 3, 512], BF16, tag="xt")
                    [nc.sync, nc.scalar][t % 2].dma_start(
                        xt_t[:],
                        xt_d.ap()[:, b * N + t * 512:b * N + (t + 1) * 512]
                        .rearrange("(c p) q -> p c q", p=128))
                    if qkvpd:
                        ps = pd_pool.tile([128, 512], F32, tag="pd", name="psq")
                    else:
                        ps = po_pool.tile([128, 512], F32, tag="po", name="psq")
                    for kc in range(3):
                        nc.tensor.matmul(ps[:], wqkv_sb[:, kc, :], xt_t[:, kc, :],
                                         start=(kc == 0), stop=(kc == 2))
                    nc.vector.tensor_copy(
                        QK4[:, b * N + t * 512:b * N + (t + 1) * 512], ps[:])
                    if vmode == "mm":
                        pa_t = pa_pool.tile([128, 384], F32, tag="pa")
                        pv = pa_t[:, 0:128].rearrange("p (a c) -> p a c", a=4)
                        for blk in range(4):
                            for kc in range(3):
                                nc.tensor.matmul(
                                    pv[:, blk, :],
                                    xt_t[:, kc, blk * 128:(blk + 1) * 128],
                                    wv_sb[:, kc, :],
                                    start=(kc == 0), stop=(kc == 2))
                        nc.vector.tensor_copy(
                            vv[:, b, t * 4:(t + 1) * 4, 0:32], pv[:])
                r = slice(b * N, (b + 1) * N)
                nc.scalar.dma_start(K2[0:32, r], QK4[32:64, r])
                nc.scalar.dma_start(Q2[32:64, r], QK4[0:32, r])
                # v natural layout via xbar DMA transpose (vT lives on strip 2)
                if vmode == "dmat":
                    nc.sync.dma_start(vv[:, b, :, 0:32], QK4[64:96, r],
                                      transpose=True)
                elif vmode == "dmat8":
                    for jc in range(NJC):
                        nc.sync.dma_start(
                            vv[:, b, jc, 0:32],
                            QK4[64:96, b * N + jc * 128:b * N + (jc + 1) * 128],
                            transpose=True)

            mi = [0]

            def head(b, ih):
                """QK^T (4x row-tiled) + exp + bias-multiply for one unit."""
                i0 = b * N + ih * IH
                pts = []
                for g in range(NJC // 4):
                    pd = pd_pool.tile([128, 4, 256], F32, tag="pd")
                    # bank-safety: strip must change only at PSUM bank
                    # boundaries (quarters 0,1 = bank 0; 2,3 = bank 1), so
                    # at most 2 concurrent row-strips for this tile layout.
                    ns = nstrips if nstrips is not None else (2 if row4 else 1)
                    for q in range(4):
                        jc = g * 4 + q
                        sp = q * ns // 4
                        ksrc = K2 if sp % 2 == 0 else QK4
                        qsrc = QK4 if sp % 2 == 0 else Q2
                        nc.tensor.matmul(
                            pd[:, q, :],
                            ksrc[32 * sp:32 * sp + 32,
                                 b * N + jc * 128:b * N + (jc + 1) * 128],
                            qsrc[32 * sp:32 * sp + 32, i0:i0 + IH],
                            start=True, stop=True,
                            tile_position=(32 * sp, 0))
                    es = es_pool.tile([128, 4, 256], BF16, tag="es")
                    if "exp" not in ablate:
                        nc.scalar.activation(es[:], pd[:], AF.Exp,
                                             scale=float(SCALE))
                    if "mult" in ablate:
                        pts.append(es)
                        continue
                    pt = pt_pool.tile([128, 4, 256], BF16, tag="pt")
                    mi[0] += 1
                    nc.vector.tensor_mul(pt[:], es[:],
                                   expb_sb[:, g * 4:(g + 1) * 4,
                                           ih * IH:(ih + 1) * IH])
                    pts.append(pt)
                return (b, ih, pts)

            ni = [0]

            def tail(b, ih, pts):
                """AV (2x col-tiled), denominators, out-proj, normalize+store."""
                i0 = b * N + ih * IH
                u2 = (b * NIH + ih) * 2
                pa_t = pa_pool.tile([128, 384], F32, tag="pa")
                if col2:
                    av = pa_t[:, 0:128]
                    for jc in range(NJC if "av" not in ablate else 0):
                        p = pts[jc // 4][:, jc % 4, :]
                        nc.tensor.matmul(av[0:avw, :], vv[:, b, jc, 0:avw],
                                         p[:, 0:128],
                                         start=(jc == 0), stop=(jc == NJC - 1),
                                         skip_group_check=True,
                                         tile_position=(0, 0))
                        nc.tensor.matmul(av[64:64 + avw, :], vv[:, b, jc, 0:avw],
                                         p[:, 128:256],
                                         start=(jc == 0), stop=(jc == NJC - 1),
                                         skip_group_check=True,
                                         tile_position=(0, 64))
                    ot = ot_pool.tile([128, 128], BF16, tag="ot")
                    if avw == 64:
                        nc.vector.tensor_copy(ot[:], av[:])
                    else:
                        nc.vector.tensor_copy(ot[0:33, :], av[0:33, :])
                        nc.vector.tensor_copy(ot[64:97, :], av[64:97, :])
                else:
                    av = pa_t[:, 0:256]
                    for jc in range(NJC):
                        p = pts[jc // 4][:, jc % 4, :]
                        nc.tensor.matmul(av[0:33, :], vv[:, b, jc, 0:33],
                                         p[:],
                                         start=(jc == 0), stop=(jc == NJC - 1))
                    ot = ot_pool.tile([128, 256], BF16, tag="ot")
                    nc.vector.tensor_copy(ot[0:33, :], av[0:33, :])

                # denominators -> natural layout -> reciprocal
                dv = pa_t[:, 256:258].bitcast(BF16)      # [128, 4] bf16
                if "dent" in ablate:
                    pass
                elif col2:
                    nc.tensor.transpose(dv[:, 0:1], ot[32:33, :],
                                        ident_sb[32:33, 0:1],
                                        tile_position=(32, 0))
                    nc.tensor.transpose(dv[:, 2:3], ot[96:97, :],
                                        ident_sb[96:97, 0:1],
                                        tile_position=(96, 0))
                else:
                    nc.tensor.transpose(dv[:, 0:1], ot[32:33, 0:128],
                                        ident_sb[32:33, 0:1],
                                        tile_position=(32, 0))
                    nc.tensor.transpose(dv[:, 2:3], ot[32:33, 128:256],
                                        ident_sb[32:33, 0:1],
                                        tile_position=(32, 0))
                if "dent" not in ablate:
                    nc.vector.reciprocal(recip_nat[:, u2:u2 + 2], dv[:, 0:4:2])
                # out projection, 2x row-tiled (strips 0 and 2)
                if "proj" in ablate:
                    return
                ob = ob_pool.tile([128, 2, OUP], BF16, tag="ob")
                for half in range(2):
                    po = po_pool.tile([128, 512], F32, tag="po")
                    if col2:
                        lhs = ot[64 * half:64 * half + 32, :]
                        w = wout_sb[64 * half:64 * half + 32, :]
                        tp = (64 * half, 0)
                    else:
                        lhs = ot[0:32, 128 * half:128 * (half + 1)]
                        w = wout_sb[0:32, :]
                        tp = (0, 0)
                    nc.tensor.matmul(po[:, 0:OUP], lhs, w,
                                     start=True, stop=True,
                                     tile_position=tp)
                    rc = recip_nat[:, u2 + half:u2 + half + 1]
                    if "norm" in ablate:
                        continue
                    if ni[0] % 8 >= 6:
                        nc.scalar.activation(ob[:, half, :], po[:, 0:OUP],
                                             AF.Copy, scale=rc)
                    else:
                        nc.vector.tensor_scalar_mul(ob[:, half, :],
                                                    po[:, 0:OUP], rc)
                    ni[0] += 1
                if "store" not in ablate and "norm" not in ablate:
                    [nc.sync, nc.scalar][(b * NIH + ih) % 2].dma_start(
                        outp_d.ap()[i0:i0 + IH, :].rearrange(
                            "(d p) f -> p d f", p=128),
                        ob[:])

            q = deque()
            for b in range(NB):
                stage_a(b)
                for ih in range(NIH):
                    if "head" in ablate:
                        continue
                    q.append(head(b, ih))
                    if "tail" in ablate:
                        q.popleft()
                        continue
                    if len(q) >= lag:
                        tail(*q.popleft())
            while q:
                tail(*q.popleft())
    nc.compile()
    return nc


def host_prep(x, w_qkv, relative_bias_table, relative_index, w_out, NB, N):
    """Build per-core input maps."""
    bf = ml_dtypes.bfloat16
    TOK = NB * N
    NJC = N // 128
    xt = np.ascontiguousarray(x.reshape(TOK, INP).T).astype(bf)
    ident = np.tile(np.eye(32, dtype=np.float32), (4, 1)).astype(bf)
    bias_full = relative_bias_table[relative_index]  # [N, N, H]
    in_maps = []
    for h in range(HEADS):
        wq = w_qkv[:, h * D:(h + 1) * D]
        wk = w_qkv[:, 256 + h * D:256 + (h + 1) * D]
        wv = w_qkv[:, 512 + h * D:512 + (h + 1) * D]
        wqkv4 = np.ascontiguousarray(
            np.concatenate([wq, wk, wv, wq], axis=1).reshape(3, 128, 128)
        ).astype(bf)
        wv3 = np.ascontiguousarray(wv.reshape(3, 128, 32)).astype(bf)
        wout4 = np.tile(w_out[h * D:(h + 1) * D, :], (4, 1)).astype(bf)
        expbT = np.exp(bias_full[:, :, h].T)  # [j, i]
        expb = np.ascontiguousarray(
            expbT.reshape(NJC, 128, N).transpose(1, 0, 2)).astype(bf)
        in_maps.append({
            "xt": xt, "wqkv4": wqkv4, "wv3": wv3, "wout4": wout4,
            "expb": expb, "ident": ident,
        })
    return in_maps


_NC_CACHE = {}


def kernel(x, w_qkv, relative_bias_table, w_out, b_out, relative_index):
    x = np.asarray(x, dtype=np.float32)
    w_qkv = np.asarray(w_qkv, dtype=np.float32)
    relative_bias_table = np.asarray(relative_bias_table, dtype=np.float32)
    w_out = np.asarray(w_out, dtype=np.float32)
    b_out = np.asarray(b_out, dtype=np.float32)
    relative_index = np.asarray(relative_index)

    NB, N, _ = x.shape
    key = (NB, N)
    if key not in _NC_CACHE:
        _NC_CACHE[key] = build_kernel(NB=NB, N=N, num_devices=HEADS)
    nc = _NC_CACHE[key]

    in_maps = host_prep(x, w_qkv, relative_bias_table, relative_index, w_out, NB, N)
    res = run_bass_kernel_spmd(nc, in_maps, core_ids=list(range(HEADS)))
    out = np.zeros((NB * N, OUP), np.float32)
    for r in res.results:
        out += r["outp"].astype(np.float32)
    out += b_out[None, :]
    return out.reshape(NB, N, OUP)


# revision 34
# speedup vs baseline: 2.0538x; 2.0538x over previous
"""Trainium2 Bass kernel for nn_Attention_86672440033867 (relative-position attention).

Sharding: head-parallel over 8 NeuronCores (1 head per core, all 16 batches).

v2 design (PE-array tiling + batch-level software pipeline):
  - QKV pass: stationary [wq|wk|wq|wk] -> psum [q,k,q,k] rows -> QK4 sbuf.
    4 cross-partition DMAs/batch build K2 (k on strips 0,2) and Q2 (q on
    strips 1,3) so every 32-row strip holds a (K-stationary, Q-moving) pair.
  - V natural: stationary x^T chunks, moving w_v -> V in [token, d] layout
    (no PE transposes of v).
  - QK^T: 4x row-tiled (tile_position auto from base partitions): 4 key
    chunks computed concurrently, contraction=32 each.  ~3x faster.
  - exp on ACT from psum [128, 4x256] (bias applied multiplicatively:
    P = exp(s*S) * exp(B), exp(B) host-precomputed, SBUF-resident).
  - multiply es*expb on DVE (2x bf16), 1 in 4 tiles on GPSIMD.
  - AV: 2x col-tiled (64-col groups): queries split in two 128-chunks, V
    carries a ones column -> softmax denominators land at psum rows 32/96.
  - denominators -> natural layout via 2 tiny PE transposes/unit -> DVE
    reciprocal (straight from psum).
  - out-proj: 2x row-tiled (strips 0 and 2 of the normalized-later OT).
  - normalize fused into the psum evacuation (tensor_scalar_mul by recip),
    split DVE/ACT; host sums the 8 partial projections and adds b_out.
"""
import numpy as np
import ml_dtypes
from collections import deque
from contextlib import ExitStack, nullcontext

import concourse.bass as bass
import concourse.mybir as mybir
import concourse.tile as tile
from concourse import bacc
from concourse.bass_utils import run_bass_kernel_spmd

BF16 = mybir.dt.bfloat16
F32 = mybir.dt.float32

HEADS = 8
D = 32          # head dim
INP = 384
OUP = 384
SCALE = D ** -0.5
AF = mybir.ActivationFunctionType


def build_kernel(NB=16, N=1024, num_devices=8, loop_k=0, row4=True, col2=False,
                 nstrips=None, ablate=(), vmode="mm", avw=33, lag=3, qkvpd=0):
    """Build the per-core Bass module. NB = total batches, N = tokens/batch."""
    assert N % 256 == 0
    NJC = N // 128          # key chunks (128) per batch
    IH = 256                # queries per attention unit
    NIH = N // IH           # units per batch
    TOK = NB * N

    nc = bacc.Bacc("TRN2", target_bir_lowering=False, num_devices=num_devices)

    xt_d = nc.dram_tensor("xt", [INP, TOK], BF16, kind="ExternalInput")
    wqkv4_d = nc.dram_tensor("wqkv4", [3, 128, 128], BF16, kind="ExternalInput")
    wv3_d = nc.dram_tensor("wv3", [3, 128, 32], BF16, kind="ExternalInput")
    wout4_d = nc.dram_tensor("wout4", [128, OUP], BF16, kind="ExternalInput")
    expb_d = nc.dram_tensor("expb", [128, NJC, N], BF16, kind="ExternalInput")
    ident_d = nc.dram_tensor("ident", [128, 32], BF16, kind="ExternalInput")
    outp_d = nc.dram_tensor("outp", [TOK, OUP], BF16, kind="ExternalOutput")

    with tile.TileContext(nc) as tc, ExitStack() as ctx:
        const = ctx.enter_context(tc.tile_pool(name="const", bufs=1))
        big = ctx.enter_context(tc.tile_pool(name="big", bufs=1))

        wqkv_sb = const.tile([128, 3, 128], BF16)
        wv_sb = const.tile([128, 3, 32], BF16)
        wout_sb = const.tile([128, OUP], BF16)
        ident_sb = const.tile([128, 32], BF16)
        expb_sb = const.tile([128, NJC, N], BF16)
        for kc in range(3):
            nc.sync.dma_start(wqkv_sb[:, kc, :], wqkv4_d.ap()[kc])
            nc.sync.dma_start(wv_sb[:, kc, :], wv3_d.ap()[kc])
        nc.sync.dma_start(wout_sb[:], wout4_d.ap())
        nc.sync.dma_start(ident_sb[:], ident_d.ap())
        nc.sync.dma_start(expb_sb[:], expb_d.ap())

        # Resident activation layouts
        QK4 = big.tile([128, TOK], BF16)     # strips 0,2: qT ; strips 1,3: kT
        K2 = big.tile([128, TOK], BF16)      # kT on strips 0,2 (DMA-replicated)
        Q2 = big.tile([128, TOK], BF16)      # qT on strips 1,3 (DMA-replicated)
        V_sb = big.tile([128, NB * NJC * 64], BF16)  # v natural + ones cols
        recip_nat = big.tile([128, NB * NIH * 2], F32)

        nc.gpsimd.memset(V_sb[:], 1.0)  # ones column pre-fill

        xt_pool = ctx.enter_context(tc.tile_pool(name="xt", bufs=6))
        es_pool = ctx.enter_context(tc.tile_pool(name="es", bufs=4))
        pt_pool = ctx.enter_context(tc.tile_pool(name="pt", bufs=6))
        ot_pool = ctx.enter_context(tc.tile_pool(name="ot", bufs=4))
        ob_pool = ctx.enter_context(tc.tile_pool(name="ob", bufs=3))

        vv = V_sb[:].rearrange("p (b j e) -> p b j e", j=NJC, e=64)

        loopB = tc.For_i(0, loop_k, 1) if loop_k else nullcontext()
        with tc.tile_pool(name="pd", bufs=2, space="PSUM") as pd_pool, \
             tc.tile_pool(name="pa", bufs=2, space="PSUM") as pa_pool, \
             tc.tile_pool(name="po", bufs=2, space="PSUM") as po_pool, loopB:

            def stage_a(b):
                """QKV projections, V-natural, replication DMAs for batch b."""
                for t in range(N // 512):
                    xt_t = xt_pool.tile([128, 3, 512], BF16, tag="xt")
                    nc.sync.dma_start(
                        xt_t[:],
                        xt_d.ap()[:, b * N + t * 512:b * N + (t + 1) * 512]
                        .rearrange("(c p) q -> p c q", p=128))
                    if qkvpd:
                        ps = pd_pool.tile([128, 512], F32, tag="pd", name="psq")
                    else:
                        ps = po_pool.tile([128, 512], F32, tag="po", name="psq")
                    for kc in range(3):
                        nc.tensor.matmul(ps[:], wqkv_sb[:, kc, :], xt_t[:, kc, :],
                                         start=(kc == 0), stop=(kc == 2))
                    nc.vector.tensor_copy(
                        QK4[:, b * N + t * 512:b * N + (t + 1) * 512], ps[:])
                    if vmode == "mm":
                        pa_t = pa_pool.tile([128, 384], F32, tag="pa")
                        pv = pa_t[:, 0:128].rearrange("p (a c) -> p a c", a=4)
                        for blk in range(4):
                            for kc in range(3):
                                nc.tensor.matmul(
                                    pv[:, blk, :],
                                    xt_t[:, kc, blk * 128:(blk + 1) * 128],
                                    wv_sb[:, kc, :],
                                    start=(kc == 0), stop=(kc == 2))
                        nc.vector.tensor_copy(
                            vv[:, b, t * 4:(t + 1) * 4, 0:32], pv[:])
                r = slice(b * N, (b + 1) * N)
                nc.gpsimd.dma_start(K2[0:32, r], QK4[32:64, r])
                nc.gpsimd.dma_start(Q2[32:64, r], QK4[0:32, r])
                # v natural layout via xbar DMA transpose (vT lives on strip 2)
                if vmode == "dmat":
                    nc.sync.dma_start(vv[:, b, :, 0:32], QK4[64:96, r],
                                      transpose=True)
                elif vmode == "dmat8":
                    for jc in range(NJC):
                        nc.sync.dma_start(
                            vv[:, b, jc, 0:32],
                            QK4[64:96, b * N + jc * 128:b * N + (jc + 1) * 128],
                            transpose=True)

            mi = [0]

            def head(b, ih):
                """QK^T (4x row-tiled) + exp + bias-multiply for one unit."""
                i0 = b * N + ih * IH
                es = es_pool.tile([128, 8, 256], BF16, tag="es")
                for g in range(NJC // 4):
                    pd = pd_pool.tile([128, 4, 256], F32, tag="pd")
                    # bank-safety: strip must change only at PSUM bank
                    # boundaries (quarters 0,1 = bank 0; 2,3 = bank 1), so
                    # at most 2 concurrent row-strips for this tile layout.
                    ns = nstrips if nstrips is not None else (2 if row4 else 1)
                    for q in range(4):
                        jc = g * 4 + q
                        sp = q * ns // 4
                        ksrc = K2 if sp % 2 == 0 else QK4
                        qsrc = QK4 if sp % 2 == 0 else Q2
                        nc.tensor.matmul(
                            pd[:, q, :],
                            ksrc[32 * sp:32 * sp + 32,
                                 b * N + jc * 128:b * N + (jc + 1) * 128],
                            qsrc[32 * sp:32 * sp + 32, i0:i0 + IH],
                            start=True, stop=True,
                            tile_position=(32 * sp, 0))
                    if "exp" not in ablate:
                        nc.scalar.activation(es[:, g * 4:(g + 1) * 4, :],
                                             pd[:], AF.Exp,
                                             scale=float(SCALE))
                if "mult" in ablate:
                    return (b, ih, es)
                pt = pt_pool.tile([128, 8, 256], BF16, tag="pt")
                eng = nc.gpsimd if (mi[0] % 4 == 3) else nc.vector
                mi[0] += 1
                eng.tensor_mul(pt[:], es[:],
                               expb_sb[:, :, ih * IH:(ih + 1) * IH])
                return (b, ih, pt)

            ni = [0]
            obs = [None]

            def tail(b, ih, pt):
                """AV (2x col-tiled), denominators, out-proj, normalize+store."""
                i0 = b * N + ih * IH
                u2 = (b * NIH + ih) * 2
                pa_t = pa_pool.tile([128, 384], F32, tag="pa")
                if col2:
                    av = pa_t[:, 0:128]
                    for jc in range(NJC if "av" not in ablate else 0):
                        p = pt[:, jc, :]
                        nc.tensor.matmul(av[0:avw, :], vv[:, b, jc, 0:avw],
                                         p[:, 0:128],
                                         start=(jc == 0), stop=(jc == NJC - 1),
                                         skip_group_check=True,
                                         tile_position=(0, 0))
                        nc.tensor.matmul(av[64:64 + avw, :], vv[:, b, jc, 0:avw],
                                         p[:, 128:256],
                                         start=(jc == 0), stop=(jc == NJC - 1),
                                         skip_group_check=True,
                                         tile_position=(0, 64))
                    ot = ot_pool.tile([128, 128], BF16, tag="ot")
                    if avw == 64:
                        nc.vector.tensor_copy(ot[:], av[:])
                    else:
                        nc.vector.tensor_copy(ot[0:33, :], av[0:33, :])
                        nc.vector.tensor_copy(ot[64:97, :], av[64:97, :])
                else:
                    av = pa_t[:, 0:256]
                    for jc in range(NJC):
                        p = pt[:, jc, :]
                        nc.tensor.matmul(av[0:33, :], vv[:, b, jc, 0:33],
                                         p[:],
                                         start=(jc == 0), stop=(jc == NJC - 1))
                    ot = ot_pool.tile([128, 256], BF16, tag="ot")
                    nc.vector.tensor_copy(ot[0:33, :], av[0:33, :])

                # denominators -> natural layout -> reciprocal
                dv = pa_t[:, 256:258].bitcast(BF16)      # [128, 4] bf16
                if "dent" in ablate:
                    pass
                elif col2:
                    nc.tensor.transpose(dv[:, 0:1], ot[32:33, :],
                                        ident_sb[32:33, 0:1],
                                        tile_position=(32, 0))
                    nc.tensor.transpose(dv[:, 2:3], ot[96:97, :],
                                        ident_sb[96:97, 0:1],
                                        tile_position=(96, 0))
                else:
                    nc.tensor.transpose(dv[:, 0:1], ot[32:33, 0:128],
                                        ident_sb[32:33, 0:1],
                                        tile_position=(32, 0))
                    nc.tensor.transpose(dv[:, 2:3], ot[32:33, 128:256],
                                        ident_sb[32:33, 0:1],
                                        tile_position=(32, 0))
                if "dent" not in ablate:
                    nc.vector.reciprocal(recip_nat[:, u2:u2 + 2], dv[:, 0:4:2])
                # out projection, 2x row-tiled (strips 0 and 2)
                if "proj" in ablate:
                    return
                u = b * NIH + ih
                if u % 2 == 0:
                    obs[0] = ob_pool.tile([128, 2, 2, OUP], BF16, tag="ob", name="ob")
                ob = obs[0][:, u % 2]
                for half in range(2):
                    po = po_pool.tile([128, 512], F32, tag="po")
                    if col2:
                        lhs = ot[64 * half:64 * half + 32, :]
                        w = wout_sb[64 * half:64 * half + 32, :]
                        tp = (64 * half, 0)
                    else:
                        lhs = ot[0:32, 128 * half:128 * (half + 1)]
                        w = wout_sb[0:32, :]
                        tp = (0, 0)
                    nc.tensor.matmul(po[:, 0:OUP], lhs, w,
                                     start=True, stop=True,
                                     tile_position=tp)
                    rc = recip_nat[:, u2 + half:u2 + half + 1]
                    if "norm" in ablate:
                        continue
                    if ni[0] % 10 >= 7:
                        nc.scalar.activation(ob[:, half, :], po[:, 0:OUP],
                                             AF.Copy, scale=rc)
                    else:
                        nc.vector.tensor_scalar_mul(ob[:, half, :],
                                                    po[:, 0:OUP], rc)
                    ni[0] += 1
                if "store" not in ablate and "norm" not in ablate and u % 2 == 1:
                    nc.gpsimd.dma_start(
                        outp_d.ap()[i0 - IH:i0 + IH, :].rearrange(
                            "(d p) f -> p d f", p=128),
                        obs[0][:])

            q = deque()
            stage_a(0)
            stage_a(1)
            for b in range(NB):
                if b + 2 < NB:
                    stage_a(b + 2)
                for ih in range(NIH):
                    if "head" in ablate:
                        continue
                    q.append(head(b, ih))
                    if "tail" in ablate:
                        q.popleft()
                        continue
                    if len(q) >= lag:
                        tail(*q.popleft())
            while q:
                tail(*q.popleft())
    nc.compile()
    return nc


def host_prep(x, w_qkv, relative_bias_table, relative_index, w_out, NB, N):
    """Build per-core input maps."""
    bf = ml_dtypes.bfloat16
    TOK = NB * N
    NJC = N // 128
    xt = np.ascontiguousarray(x.reshape(TOK, INP).T).astype(bf)
    ident = np.tile(np.eye(32, dtype=np.float32), (4, 1)).astype(bf)
    bias_full = relative_bias_table[relative_index]  # [N, N, H]
    in_maps = []
    for h in range(HEADS):
        wq = w_qkv[:, h * D:(h + 1) * D]
        wk = w_qkv[:, 256 + h * D:256 + (h + 1) * D]
        wv = w_qkv[:, 512 + h * D:512 + (h + 1) * D]
        wqkv4 = np.ascontiguousarray(
            np.concatenate([wq, wk, wv, wq], axis=1).reshape(3, 128, 128)
        ).astype(bf)
        wv3 = np.ascontiguousarray(wv.reshape(3, 128, 32)).astype(bf)
        wout4 = np.tile(w_out[h * D:(h + 1) * D, :], (4, 1)).astype(bf)
        expbT = np.exp(bias_full[:, :, h].T)  # [j, i]
        expb = np.ascontiguousarray(
            expbT.reshape(NJC, 128, N).transpose(1, 0, 2)).astype(bf)
        in_maps.append({
            "xt": xt, "wqkv4": wqkv4, "wv3": wv3, "wout4": wout4,
            "expb": expb, "ident": ident,
        })
    return in_maps


_NC_CACHE = {}


def kernel(x, w_qkv, relative_bias_table, w_out, b_out, relative_index):
    x = np.asarray(x, dtype=np.float32)
    w_qkv = np.asarray(w_qkv, dtype=np.float32)
    relative_bias_table = np.asarray(relative_bias_table, dtype=np.float32)
    w_out = np.asarray(w_out, dtype=np.float32)
    b_out = np.asarray(b_out, dtype=np.float32)
    relative_index = np.asarray(relative_index)

    NB, N, _ = x.shape
    key = (NB, N)
    if key not in _NC_CACHE:
        _NC_CACHE[key] = build_kernel(NB=NB, N=N, num_devices=HEADS)
    nc = _NC_CACHE[key]

    in_maps = host_prep(x, w_qkv, relative_bias_table, relative_index, w_out, NB, N)
    res = run_bass_kernel_spmd(nc, in_maps, core_ids=list(range(HEADS)))
    out = np.zeros((NB * N, OUP), np.float32)
    for r in res.results:
        out += r["outp"].astype(np.float32)
    out += b_out[None, :]
    return out.reshape(NB, N, OUP)
